# revision 49
# baseline (speedup 1.0000x reference)
"""Trainium2 Bass kernel for pre-LN multi-head attention (B=2, S=2048, H=1024, 16 heads).

Sharding: 8 cores = 2 batches x 4 query-blocks of 512 rows (no collectives).
All matmuls run in fp8e4m3 DoubleRow (2 contraction tiles per pass, 0.5
cycles/row). LayerNorm: x^T ships as bf16; ynT8 = fp8(x * rstd); the -mu
correction rides as a K=1 matmul using ms = fp8(-mu*rstd*SM) against host-
provided colsum rows fp8(colsum(W8)/SM). Weights are prescaled by WS=512 on
the host, un-scaled at PSUM evacuation. rstd = exp(-0.5*ln(var+eps)) so the
whole kernel uses a single activation table set (ln/exp/square). Softmax:
exp over [128,1024] PSUM score regions straight to fp8; denominator via an
appended ones column on V. Pair 0's attention is interleaved with the
remaining LayerNorm quarters so the Act-engine exp storm starts at ~10us.
"""

import sys
import numpy as np
from contextlib import ExitStack

sys.path.insert(0, "/opt/trn_rl_repo")

import ml_dtypes  # noqa: E402
import concourse.bass as bass  # noqa: E402
import concourse.bacc as bacc  # noqa: E402
import concourse.tile as tile  # noqa: E402
from concourse import mybir  # noqa: E402

B, S, H = 2, 2048, 1024
HEADS, HD = 16, 64
NCORES = 8
SQ = 512          # query rows per core
HT = H // 128     # 8 hidden tiles
PAIRS = HEADS // 2
KCH = S // 128    # 16 key chunks of 128
WS = 512.0        # weight prescale (power of two, exact)
SM = 64.0         # correction-row scale split
F32 = mybir.dt.float32
F32R = mybir.dt.float32r
BF16 = mybir.dt.bfloat16
F8 = mybir.dt.float8e4
AF = mybir.ActivationFunctionType
OP = mybir.AluOpType
DRM = mybir.MatmulPerfMode.DoubleRow
E4 = ml_dtypes.float8_e4m3
BF = ml_dtypes.bfloat16


def slot0(ap):
    """[p, n] AP -> [p, 2, n] AP with stride-0 slot dim (reads data twice)."""
    return bass.AP(tensor=ap.tensor, offset=ap.offset,
                   ap=[list(ap.ap[0])] + [[0, 2]] + [list(d) for d in ap.ap[1:]])


def build_nc():
    nc = bacc.Bacc()
    xT = nc.dram_tensor("xT", [H, S], F8, kind="ExternalInput")
    xrb = nc.dram_tensor("xrb", [SQ, H], F32, kind="ExternalInput")
    wq8 = nc.dram_tensor("wq8", [8, 128, HT, 128], F8, kind="ExternalInput")
    wk8 = nc.dram_tensor("wk8", [8, 128, HT, 128], F8, kind="ExternalInput")
    wv8 = nc.dram_tensor("wv8", [2, 128, HT, 512], F8, kind="ExternalInput")
    wo8 = nc.dram_tensor("wo8", [128, HT, H], F8, kind="ExternalInput")
    cs8 = nc.dram_tensor("cs8", [3, H], F8, kind="ExternalInput")
    bq = nc.dram_tensor("bq", [H], F32, kind="ExternalInput")
    bk = nc.dram_tensor("bk", [H], F32, kind="ExternalInput")
    bv = nc.dram_tensor("bv", [H], F32, kind="ExternalInput")
    out = nc.dram_tensor("out", [SQ, H], F32, kind="ExternalOutput")

    xT_t = xT[:, :].rearrange("(t p) s -> p t s", p=128)        # [128, 8, 2048]

    def colvec(v):
        return v[:].rearrange("(t p) -> p t", p=128)

    inv_h = 1.0 / H

    with tile.TileContext(nc) as tc, ExitStack() as ctx:
        persist = ctx.enter_context(tc.tile_pool(name="persist", bufs=1))
        stream = ctx.enter_context(tc.tile_pool(name="stream", bufs=1))
        psum = ctx.enter_context(tc.tile_pool(name="psum", bufs=1, space="PSUM"))

        # ---- persistent sbuf ----
        ynT8 = persist.tile([128, HT, S], F8)
        ms8 = persist.tile([1, S], F8)              # -mu*rstd*SM correction row
        rstd_bc = persist.tile([128, S], F32)
        qt8 = persist.tile([128, PAIRS, SQ], F8)
        v8 = persist.tile([128, KCH, HEADS * 65], F8)
        ctxT8 = persist.tile([128, HT, SQ], F8)
        kt8 = [persist.tile([128, 2, S], F8, name=f"kt8_{i}") for i in range(2)]
        wq8s = persist.tile([128, 8, HT, 128], F8)
        wk8s = persist.tile([128, 8, HT, 128], F8)
        wv8s = persist.tile([128, 2, HT, 512], F8)
        wo8s = persist.tile([128, HT, H], F8)
        csq8s = persist.tile([1, H], F8)
        csk8s = persist.tile([1, H], F8)
        csv8s = persist.tile([1, H], F8)
        bcols = persist.tile([128, 2 * HT], F32)
        bv_row = persist.tile([1, H], F32)
        bv_bc = persist.tile([128, H], F32)
        ones_bf = persist.tile([128, 1], F8)
        eps_t = persist.tile([1, 1], F32)
        dummy = persist.tile([1, 1], F32)

        # ---- small setup (bulk x/weight transfers get SP queue priority;
        # small tensors ride the gpsimd SWDGE queue) ----
        nc.vector.memset(ones_bf, 1.0)
        nc.vector.memset(eps_t, 1e-5)
        # single activation-table load for the whole kernel (ln/exp/square)
        nc.scalar.activation(out=dummy, in_=eps_t, func=AF.Ln)
        v8_j = v8.rearrange("p k (j c) -> p k j c", c=65)

        # ---- SP DMA sequencing: transfers execute in emission order ----
        xq = [stream.tile([128, HT, 512], F8, tag="xq", bufs=4, name="xq")
              for _ in range(4)]

        def dma_x(q):
            sl = slice(q * 512, (q + 1) * 512)
            nc.sync.dma_start(out=xq[q][:, 0:4, :], in_=xT_t[:, 0:4, sl])
            nc.sync.dma_start(out=xq[q][:, 4:8, :], in_=xT_t[:, 4:8, sl])

        def dma_wchunk(w8s, wdram, c):
            nc.sync.dma_start(out=w8s[:, c, :, :], in_=wdram[c, :, :, :])

        prim = stream.tile([128, 512], F8, tag="xsq", bufs=4, name="prim")
        nc.vector.memset(prim, 0.0)
        pacc = psum.tile([1, 512], F32, tag="acc", bufs=1, name="pacc")
        for i in range(10):
            nc.tensor.matmul(pacc, ones_bf, prim, start=(i == 0),
                             stop=(i == 9), skip_group_check=True)
        dma_x(0)
        dma_wchunk(wq8s, wq8, 0)
        dma_wchunk(wk8s, wk8, 0)
        nc.sync.dma_start(out=wv8s[:, 0, :, :], in_=wv8[0, :, :, :])
        nc.scalar.dma_start(out=bv_row, in_=bv[:].rearrange("(o d) -> o d", o=1))
        nc.scalar.dma_start(out=bcols[:, 0:HT], in_=colvec(bq))
        nc.scalar.dma_start(out=bcols[:, HT:2 * HT], in_=colvec(bk))
        nc.scalar.dma_start(out=csq8s, in_=cs8[0:1, :])
        nc.scalar.dma_start(out=csk8s, in_=cs8[1:2, :])
        nc.scalar.dma_start(out=csv8s, in_=cs8[2:3, :])
        bqcol, bkcol = bcols[:, 0:HT], bcols[:, HT:2 * HT]

        # ---- phase 0 pieces ----
        def quarter(q, spool):
            sl = slice(q * 512, (q + 1) * 512)
            st = spool.tile([33, 512], F32, tag="stat", bufs=1, name="stat")
            sacc, qacc = st[0:1, :], st[32:33, :]
            for h in range(HT):
                nc.tensor.matmul(sacc, ones_bf, xq[q][:, h, :],
                                 start=(h == 0), stop=(h == HT - 1),
                                 skip_group_check=True)
            xsqs = []
            for h in range(HT):
                xsq = stream.tile([128, 512], F8, tag="xsq", bufs=4, name="xsq")
                nc.gpsimd.tensor_mul(xsq, xq[q][:, h, :], xq[q][:, h, :])
                xsqs.append(xsq)
            for h in range(HT):
                nc.tensor.matmul(qacc, ones_bf, xsqs[h],
                                 start=(h == 0), stop=(h == HT - 1),
                                 skip_group_check=True)
            # evacuate stat rows to SBUF once; epilogue runs on gpsimd
            m_ = stream.tile([1, 512], F32, tag="srow", bufs=1, name="m_")
            var = stream.tile([1, 512], F32, tag="var", bufs=1, name="var")
            rstd = stream.tile([1, 512], F32, tag="rstd", bufs=2, name="rstd")
            nc.vector.tensor_scalar_mul(m_, sacc, inv_h)       # mean
            nc.vector.scalar_tensor_tensor(out=var, in0=m_, scalar=-1.0,
                                           in1=m_, op0=OP.mult, op1=OP.mult)
            nc.vector.scalar_tensor_tensor(out=var, in0=qacc, scalar=inv_h,
                                           in1=var, op0=OP.mult, op1=OP.add)
            # rstd = exp(-0.5 * ln(var + eps)): stays in the ln/exp table set
            nc.scalar.activation(out=var, in_=var, func=AF.Ln, bias=eps_t[:])
            nc.scalar.activation(out=rstd, in_=var, func=AF.Exp, scale=-0.5)
            nc.vector.scalar_tensor_tensor(out=ms8[0:1, sl], in0=m_,
                                           scalar=-SM, in1=rstd,
                                           op0=OP.mult, op1=OP.mult)
            nc.gpsimd.partition_broadcast(rstd_bc[:, sl], rstd)
            ndve = 4 if q == 0 else 3
            for h in range(HT):
                peng = nc.vector if h < ndve else nc.gpsimd
                peng.tensor_mul(ynT8[:, h, sl], xq[q][:, h, :], rstd_bc[:, sl])

        def q_group(t):
            acc = psum.tile([128, 512], F32, tag="acc", bufs=1, name="acc_q")
            for i in range(4):
                nc.tensor.matmul(acc, wq8s[:, t, 2 * i:2 * i + 2, :],
                                 ynT8[:, 2 * i:2 * i + 2, 0:512],
                                 start=(i == 0), stop=False, perf_mode=DRM)
            nc.tensor.matmul(acc, csq8s[:, t * 128:(t + 1) * 128],
                             ms8[0:1, 0:512], start=False, stop=True)
            nc.vector.tensor_scalar(out=qt8[:, t, :], in0=acc,
                                    scalar1=1.0 / WS, scalar2=bqcol[:, t:t + 1],
                                    op0=OP.mult, op1=OP.add)

        def k_group(pair, q, kbuf):
            sl = slice(q * 512, (q + 1) * 512)
            acc = psum.tile([128, 512], F32, tag="acc", bufs=1, name="acc_k")
            for i in range(4):
                nc.tensor.matmul(acc, wk8s[:, pair, 2 * i:2 * i + 2, :],
                                 ynT8[:, 2 * i:2 * i + 2, sl],
                                 start=(i == 0), stop=False, perf_mode=DRM)
            nc.tensor.matmul(acc, csk8s[:, pair * 128:(pair + 1) * 128],
                             ms8[0:1, sl], start=False, stop=True)
            nc.vector.tensor_scalar(out=kt8[kbuf][:, 0, sl], in0=acc,
                                    scalar1=1.0 / WS,
                                    scalar2=bkcol[:, pair:pair + 1],
                                    op0=OP.mult, op1=OP.add)

        def v_group(kc, jh):
            ksl = slice(kc * 128, (kc + 1) * 128)
            acc = psum.tile([128, 512], F32, tag="acc", bufs=1, name="acc_v")
            for i in range(4):
                nc.tensor.matmul(acc, ynT8[:, 2 * i:2 * i + 2, ksl],
                                 wv8s[:, jh, 2 * i:2 * i + 2, :],
                                 start=(i == 0), stop=False, perf_mode=DRM)
            nc.tensor.matmul(acc, ms8[0:1, ksl],
                             csv8s[:, jh * 512:(jh + 1) * 512],
                             start=False, stop=True)
            nc.vector.scalar_tensor_tensor(
                out=v8_j[:, kc, 8 * jh:8 * jh + 8, 0:64], in0=acc,
                scalar=1.0 / WS, in1=bv_bc[:, jh * 512:(jh + 1) * 512],
                op0=OP.mult, op1=OP.add)

        # ---- attention head machinery (supports interleaved emission) ----
        rpool = ctx.enter_context(tc.tile_pool(name="regpool", bufs=2,
                                               space="PSUM"))
        spool = ctx.enter_context(tc.tile_pool(name="statps", bufs=1,
                                               space="PSUM"))


        class Head:
            def __init__(self, j, pair, kbuf):
                self.j, self.pair, self.kbuf = j, pair, kbuf
                self.po = 64 * (j % 2)
                self.cps = psum.tile([65, 512], F32, tag="ctx", bufs=2,
                                     name="cps")
                self.qmov = slot0(qt8[self.po:self.po + 64, pair, :])
                self.pend = []

            def scores_exp(self, reg):
                kc0 = 2 * reg
                po = self.po
                region = rpool.tile([128, 1024], F32, tag="region", name="reg")
                nc.tensor.matmul(
                    region[:, 0:512],
                    kt8[self.kbuf][po:po + 64, :, kc0 * 128:(kc0 + 1) * 128],
                    self.qmov, start=True, stop=True, perf_mode=DRM)
                nc.tensor.matmul(
                    region[:, 512:1024],
                    kt8[self.kbuf][po:po + 64, :, (kc0 + 1) * 128:(kc0 + 2) * 128],
                    self.qmov, start=True, stop=True, perf_mode=DRM)
                et = stream.tile([128, 2, 512], F8, tag="et", bufs=4, name="et")
                nc.scalar.activation(out=et, in_=region, func=AF.Exp, scale=0.125)
                self.pend.append((reg, et))

            def ctx_dr(self):
                reg, et = self.pend.pop(0)
                nc.tensor.matmul(self.cps,
                                 v8[:, 2 * reg:2 * reg + 2,
                                    self.j * 65:self.j * 65 + 65],
                                 et, start=(reg == 0), stop=(reg == 7),
                                 perf_mode=DRM)

            def evac(self):
                while self.pend:
                    self.ctx_dr()
                recip = stream.tile([1, 512], F32, tag="recip", bufs=2,
                                    name="recip")
                nc.vector.reciprocal(out=recip, in_=self.cps[64:65, :])
                rbc = stream.tile([64, 512], F32, tag="rbc", bufs=2, name="rbc")
                nc.gpsimd.partition_broadcast(rbc, recip)
                nc.vector.tensor_mul(ctxT8[self.po:self.po + 64, self.pair, :],
                                     self.cps[0:64, :], rbc)

        # --- pair 0 interleaved with the LayerNorm quarters ---
        quarter(0, spool)
        nc.gpsimd.partition_broadcast(bv_bc, bv_row)
        nc.gpsimd.memset(kt8[0][:, 1, :], 0.0)   # DR slot-1 zeros (stay zero)
        nc.gpsimd.memset(kt8[1][:, 1, :], 0.0)
        nc.gpsimd.memset(v8_j[:, :, :, 64:65], 1.0)  # softmax-denominator ones
        dma_x(1)
        dma_wchunk(wq8s, wq8, 1)
        dma_wchunk(wk8s, wk8, 1)
        q_group(0)
        h0 = Head(0, 0, 0)
        h1 = Head(1, 0, 0)

        def p0_quarter(q):
            # scores+exp first (they only need K0(q)); V groups and the
            # deferred ctx-DRs follow, so the Act engine never waits on V
            k_group(0, q, 0)
            for r in (2 * q, 2 * q + 1):
                h0.scores_exp(r)
                h1.scores_exp(r)
            for kc in range(4 * q, 4 * q + 4):
                v_group(kc, 0)
            for _ in range(2):
                h0.ctx_dr()
                h1.ctx_dr()

        p0_quarter(0)
        quarter(1, spool)
        dma_x(2)
        for c in (2, 3):
            dma_wchunk(wq8s, wq8, c)
            dma_wchunk(wk8s, wk8, c)
        p0_quarter(1)
        k_group(1, 0, 1)
        quarter(2, spool)
        dma_x(3)
        p0_quarter(2)
        k_group(1, 1, 1)
        quarter(3, spool)
        for c in (4, 5, 6, 7):
            dma_wchunk(wq8s, wq8, c)
            dma_wchunk(wk8s, wk8, c)
        nc.sync.dma_start(out=wv8s[:, 1, :, :], in_=wv8[1, :, :, :])
        nc.sync.dma_start(out=wo8s, in_=wo8[:, :, :])
        p0_quarter(3)
        k_group(1, 2, 1)
        k_group(1, 3, 1)
        h0.evac()
        h1.evac()
        q_group(1)

        # --- pairs 1-7 with spread side-work ---
        xr_tiles = {}
        ostash = {}

        def oproj_partial(qc, jh):
            dsl = slice(jh * 512, (jh + 1) * 512)
            acc = psum.tile([128, 512], F32, tag="acc", bufs=1, name="acc_op")
            for i in range(3):
                nc.tensor.matmul(acc, ctxT8[:, 2 * i:2 * i + 2,
                                            qc * 128:(qc + 1) * 128],
                                 wo8s[:, 2 * i:2 * i + 2, dsl],
                                 start=(i == 0), stop=(i == 2),
                                 perf_mode=DRM)
            g = 2 * qc + jh
            nc.vector.scalar_tensor_tensor(out=ostash[g], in0=acc,
                                           scalar=1.0 / WS, in1=xr_tiles[g],
                                           op0=OP.mult, op1=OP.add)

        def head_run(j, pair, kbuf, work):
            hd = Head(j, pair, kbuf)
            for reg in range(8):
                hd.scores_exp(reg)
                if reg >= 1:
                    hd.ctx_dr()
                if reg >= 1 and work:
                    work.pop(0)()
            hd.ctx_dr()
            hd.evac()

        vwork = [lambda kc=kc: v_group(kc, 1) for kc in range(KCH)]
        pwork = {t: [] for t in range(1, PAIRS)}
        for t in range(1, PAIRS - 1):
            pwork[t] += [lambda q=q, t=t: k_group(t + 1, q, (t + 1) % 2)
                         for q in range(4)]
            pwork[t].append(lambda t=t: q_group(t + 1))
        for t in (1, 2, 3):
            pwork[t] += vwork[(t - 1) * 6:(t - 1) * 6 + 6]

        owork = []
        for pair in range(1, PAIRS):
            work = owork if pair == PAIRS - 1 else pwork[pair]
            head_run(2 * pair, pair, pair % 2, work)
            head_run(2 * pair + 1, pair, pair % 2, work)
            for w in work:
                w()
            work.clear()
            if pair == 4:
                # prefetch residual tiles into recycled x-staging tiles
                for half in range(2):
                    xrt = stream.tile([128, 2, HT, 512], F8, tag="xr", bufs=2,
                                      name="xrt")
                    xrf = xrt.rearrange("p a t d -> p (a t d)").bitcast(F32)
                    xrf = xrf.rearrange("p (g d) -> p g d", d=512)
                    nc.sync.dma_start(
                        out=xrf.rearrange("p (t j) d -> p t j d", j=2),
                        in_=xrb[half * 256:(half + 1) * 256, :].rearrange(
                            "(t p) (j d) -> p t j d", p=128, d=512))
                    for s in range(4):
                        xr_tiles[half * 4 + s] = xrf[:, s, :]
            if pair == 5:
                ostash2 = {}
                for half in range(2):
                    ost = stream.tile([128, 2, HT, 512], F8, tag="ost", bufs=2,
                                      name="ost")
                    osf = ost.rearrange("p a t d -> p (a t d)").bitcast(F32)
                    os2 = osf.rearrange("p (q d) -> p q d", d=1024)
                    osf = osf.rearrange("p (g d) -> p g d", d=512)
                    for s in range(4):
                        ostash[half * 4 + s] = osf[:, s, :]
                    for s in range(2):
                        ostash2[half * 2 + s] = os2[:, s, :]
                owork += [lambda qc=qc, jh=jh: oproj_partial(qc, jh)
                          for qc in range(4) for jh in range(2)]

        # ---- output projection tail: ctx pairs 6-7 + stash + store ----
        for qc in range(4):
            reg = rpool.tile([128, 1024], F32, tag="region", name="reg_o")
            for jh in range(2):
                nc.tensor.matmul(reg[:, jh * 512:(jh + 1) * 512],
                                 ctxT8[:, 6:8, qc * 128:(qc + 1) * 128],
                                 wo8s[:, 6:8, jh * 512:(jh + 1) * 512],
                                 start=True, stop=True, perf_mode=DRM)
            osb = stream.tile([128, 1024], F32, tag="osb", bufs=2, name="osb")
            nc.vector.scalar_tensor_tensor(out=osb, in0=reg,
                                           scalar=1.0 / WS,
                                           in1=ostash2[qc],
                                           op0=OP.mult, op1=OP.add)
            eng = nc.sync if qc % 2 == 0 else nc.gpsimd
            eng.dma_start(out=out[qc * 128:(qc + 1) * 128, :], in_=osb)
    nc.finalize()
    return nc


_NC = None


def _get_nc():
    global _NC
    if _NC is None:
        _NC = build_nc()
    return _NC


def _q8(a):
    return np.asarray(a, np.float32).astype(E4)


def make_in_maps(inputs):
    x = np.asarray(inputs["x"], np.float32)
    g = np.asarray(inputs["ln_g"], np.float32)
    lnb = np.asarray(inputs["ln_b"], np.float32)
    wq = np.asarray(inputs["Wq"], np.float32)
    wk = np.asarray(inputs["Wk"], np.float32)
    wv = np.asarray(inputs["Wv"], np.float32)
    wo = np.asarray(inputs["Wo"], np.float32)

    wq8 = _q8(WS * (wq * g).T)    # [hidden, outdim]
    wk8 = _q8(WS * (wk * g).T)
    wv8 = _q8(WS * (wv * g).T)
    wo8 = _q8(WS * wo.T)
    cs8 = np.stack([_q8(w.astype(np.float32).sum(0) / SM)
                    for w in (wq8, wk8, wv8)])

    shared = {
        # chunk-major layouts so each DMA lands contiguous >=1KB runs
        "wq8": np.ascontiguousarray(
            wq8.reshape(8, 128, 8, 128).transpose(2, 1, 0, 3)),
        "wk8": np.ascontiguousarray(
            wk8.reshape(8, 128, 8, 128).transpose(2, 1, 0, 3)),
        "wv8": np.ascontiguousarray(
            wv8.reshape(8, 128, 2, 512).transpose(2, 1, 0, 3)),
        "wo8": np.ascontiguousarray(wo8.reshape(8, 128, H).transpose(1, 0, 2)),
        "cs8": cs8,
        "bq": np.asarray(inputs["bq"], np.float32) + wq @ lnb,
        "bk": np.asarray(inputs["bk"], np.float32) + wk @ lnb,
        "bv": np.asarray(inputs["bv"], np.float32) + wv @ lnb,
    }
    bo = np.asarray(inputs["bo"], np.float32)
    in_maps = []
    for c in range(NCORES):
        b, q0 = c // 4, (c % 4) * SQ
        xbT = x[b].T  # [H, S]
        m = dict(shared)
        # roll so this core's own 512 query columns come first; attention is
        # invariant to a consistent permutation of the key/value axis.
        m["xT"] = np.ascontiguousarray(np.roll(xbT, -q0, axis=1)).astype(E4)
        m["xrb"] = x[b, q0:q0 + SQ, :] + bo
        in_maps.append(m)
    return in_maps


def kernel(**inputs):
    from concourse.bass_utils import run_bass_kernel_spmd
    nc = _get_nc()
    in_maps = make_in_maps(inputs)
    res = run_bass_kernel_spmd(nc, in_maps, list(range(NCORES)))
    x = np.asarray(inputs["x"], np.float32)
    out = np.empty_like(x)
    for c in range(NCORES):
        b, q0 = c // 4, (c % 4) * SQ
        out[b, q0:q0 + SQ, :] = res.results[c]["out"]
    return out
